# revision 1
# baseline (speedup 1.0000x reference)
"""Distributed Trainium2 kernel for AttentionalPropagation (SuperGlue-style).

Reference computation (B=4, D=256, H=4, N=2048):
    q = Wq x + bq ; k = Wk s + bk ; v = Wv s + bv           (1x1 convs)
    prob = softmax(q^T k / sqrt(D))  per (b, h)
    msg  = Wm (v prob^T) + bm
    h1   = W1 [x; msg] + b1
    y    = BN(h1) * gamma + beta ; relu
    out  = W2 y + b2

Sharding: the 16 (b, h) pairs are split 2-per-core across 8 NeuronCores
(data-parallel over B x tensor-parallel over H). Attention and the 1x1-conv
GEMMs are fully independent per (b, h); the only cross-core dependency is the
BatchNorm statistics, reduced with a tiny (4 KB) AllGather + local sum.

All GEMMs run in bf16 (fp32 PSUM accumulate); the 2e-2 rel-err budget has
plenty of headroom for that.

Engine balance: TensorE does the GEMMs; ScalarE does exp / the W1 evacuation
(which needs accum_out for BN sums) / BN-apply+ReLU; VectorE does all other
PSUM evacuations, the softmax normalization, and the BN sum-of-squares.
"""

import os
import sys
from functools import partial

import numpy as np

sys.path.insert(0, "/opt/trn_rl_repo")

import concourse.bass as bass
import concourse.bacc as bacc
import concourse.tile as tile
from concourse import mybir
from concourse.bass_utils import run_bass_kernel_spmd
from concourse.masks import make_identity

import ml_dtypes

BF16 = ml_dtypes.bfloat16

B, D, H, N = 4, 256, 4, 2048
EPS = 1e-5
P = 128
NCORES = 8
PAIRS_PER_CORE = (B * H) // NCORES  # 2
CT = D // P      # channel tiles for D (2)
CT2 = 2 * D // P # channel tiles for 2D (4)
MT = N // P      # m tiles (16)
NCH = 4          # n chunks of 512
CHUNK = N // NCH # 512

AF = mybir.ActivationFunctionType
ALU = mybir.AluOpType
f32 = mybir.dt.float32
bf16 = mybir.dt.bfloat16

_CACHE = {}


def build_bass() -> bass.Bass:
    nc = bacc.Bacc("TRN2", num_devices=NCORES)

    # ---- DRAM parameters (per-core shards; weights replicated) ----
    # Layouts match the SBUF destinations exactly: one contiguous DMA each.
    xb = nc.dram_tensor("xb", [PAIRS_PER_CORE, P, CT, N], bf16, kind="ExternalInput")
    sb = nc.dram_tensor("sb", [PAIRS_PER_CORE, P, CT, N], bf16, kind="ExternalInput")
    wqT = nc.dram_tensor("wqT", [P, CT, D], bf16, kind="ExternalInput")
    wkT = nc.dram_tensor("wkT", [P, CT, D], bf16, kind="ExternalInput")
    wvT = nc.dram_tensor("wvT", [P, CT, D], bf16, kind="ExternalInput")
    wmT = nc.dram_tensor("wmT", [P, CT, D], bf16, kind="ExternalInput")
    w1T = nc.dram_tensor("w1T", [P, CT2, 2 * D], bf16, kind="ExternalInput")
    w2T = nc.dram_tensor("w2T", [P, CT2, D], bf16, kind="ExternalInput")
    vecs = nc.dram_tensor("vecs", [P, 24], f32, kind="ExternalInput")
    out = nc.dram_tensor("out", [PAIRS_PER_CORE, CT, P, N], bf16, kind="ExternalOutput")

    # bounce buffers for the BN-stats AllReduce, plus a tiny warmup
    # AllReduce issued at kernel start so the real one (on the critical
    # path between pass 1 and pass 2) hits warm ncfw state.
    cc_in = nc.dram_tensor("cc_in", [P, 2 * CT2], f32)
    cc_out = nc.dram_tensor("cc_out", [P, 2 * CT2], f32, addr_space="Shared")
    cw_in = nc.dram_tensor("cw_in", [1, 8], f32)
    cw_out = nc.dram_tensor("cw_out", [1, 8], f32, addr_space="Shared")

    with tile.TileContext(nc) as tc:
        with (
            tc.tile_pool(name="consts", bufs=1) as consts,
            tc.tile_pool(name="persist", bufs=1) as persist,
            tc.tile_pool(name="pairbuf", bufs=1) as pairbuf,
            tc.tile_pool(name="work", bufs=2) as work,
            tc.tile_pool(name="psum", bufs=6, space="PSUM") as psum,
            tc.tile_pool(name="psum_s", bufs=2, space="PSUM") as psum_s,
        ):
            # ---- load weights/constants (single DMA each; weights go on
            # the gpsimd SWDGE queue so issue overlaps the sync-queue x/s) ----
            def load_lhsT(name, dram, kt, width, engine):
                t = consts.tile([P, kt, width], bf16, tag=name, name=name)
                engine.dma_start(out=t[:], in_=dram[:])
                return t

            wq_s = load_lhsT("wq_s", wqT, CT, D, nc.sync)
            wk_s = load_lhsT("wk_s", wkT, CT, D, nc.sync)
            wv_s = load_lhsT("wv_s", wvT, CT, D, nc.gpsimd)
            wm_s = load_lhsT("wm_s", wmT, CT, D, nc.gpsimd)
            w1_s = load_lhsT("w1_s", w1T, CT2, 2 * D, nc.gpsimd)
            w2_s = load_lhsT("w2_s", w2T, CT2, D, nc.gpsimd)

            vec_s = consts.tile([P, 24], f32, tag="vec_s")
            nc.gpsimd.dma_start(out=vec_s[:], in_=vecs[:])
            bq_s = vec_s[:, 0:2]
            bk_s = vec_s[:, 2:4]
            bv_s = vec_s[:, 4:6]
            bm_s = vec_s[:, 6:8]
            b1_s = vec_s[:, 8:12]
            b2_s = vec_s[:, 12:14]
            gm_s = vec_s[:, 14:18]
            bt_s = vec_s[:, 18:22]

            # Force the natural_log/exp activation table set to load up front
            # (during the initial DMA wait) so neither the attention Exp nor
            # the BN rsqrt (= exp(-0.5 ln)) needs a mid-kernel table switch.
            warm = persist.tile([P, 1], f32, tag="warm")
            nc.vector.memset(warm, 1.0)
            nc.scalar.activation(warm, warm, AF.Ln)
            nc.scalar.activation(warm, warm, AF.Exp)

            pe_w = persist.tile([P, CHUNK], bf16, tag="pe_w")
            nc.vector.memset(pe_w, 0.0)
            for _ in range(10):
                pw = psum.tile([P, CHUNK], f32, tag="mm512", name="mmps")
                nc.tensor.matmul(pw, pe_w[:, 0:P], pe_w, start=True, stop=True)

            nc.gpsimd.collective_compute(
                "AllReduce",
                ALU.add,
                replica_groups=[list(range(NCORES))],
                ins=[cw_in[:].opt()],
                outs=[cw_out[:].opt()],
            )

            # BN partial sums: [channel-tile, slot] with one slot per
            # (pair, n-chunk) evacuation call (accum_out overwrites per call).
            nslots = PAIRS_PER_CORE * NCH
            ssq = persist.tile([P, CT2, nslots], f32, tag="ssq")
            sigma = persist.tile([P, PAIRS_PER_CORE, CT2], bf16, tag="sigma")
            h1 = [
                persist.tile([P, CT2, N], bf16, tag=f"h1_{p}", name=f"h1_{p}")
                for p in range(PAIRS_PER_CORE)
            ]

            def conv_proj(w_t, rhs_srcs, kt, m_tiles, dst_cb):
                """out[m*P:(m+1)*P, :] = sum_k w_t[:,k,mP:(m+1)P].T @ rhs_k.

                Weight-stationary: k outer, n-chunk inner, so each lhsT is
                loaded once per (k, m) instead of once per matmul.
                dst_cb(m, j, ps) evacuates one [P, CHUNK] PSUM chunk.
                """
                for m in range(m_tiles):
                    pss = [
                        psum.tile([P, CHUNK], f32, tag="mm512", name="mmps")
                        for _ in range(NCH)
                    ]
                    for k in range(kt):
                        lhsT = w_t[:, k, m * P : (m + 1) * P]
                        for j in range(NCH):
                            nc.tensor.matmul(
                                pss[j],
                                lhsT,
                                rhs_srcs[k][:, j * CHUNK : (j + 1) * CHUNK],
                                start=(k == 0),
                                stop=(k == kt - 1),
                            )
                    for j in range(NCH):
                        dst_cb(m, j, pss[j])

            for p in range(PAIRS_PER_CORE):
                # ---- load inputs (chunked so compute starts early) ----
                x_s = work.tile([P, CT, N], bf16, tag="x_s")
                s_s = work.tile([P, CT, N], bf16, tag="s_s", bufs=1)
                for hh in range(2):
                    sl = slice(hh * (N // 2), (hh + 1) * (N // 2))
                    nc.sync.dma_start(out=x_s[:, :, sl], in_=xb[p, :, :, sl])
                    nc.sync.dma_start(out=s_s[:, :, sl], in_=sb[p, :, :, sl])

                # ---- q/k projections (VectorE evacuation + bias) ----
                q_s = pairbuf.tile([P, CT, N], bf16, tag="q_s")
                k_s = pairbuf.tile([P, CT, N], bf16, tag="k_s")

                def evac_bias(m, j, ps, dst=None, b_t=None):
                    nc.vector.tensor_scalar_add(
                        dst[:, m, j * CHUNK : (j + 1) * CHUNK], ps, b_t[:, m : m + 1]
                    )

                conv_proj(
                    wq_s, [x_s[:, k, :] for k in range(CT)], CT, CT,
                    partial(evac_bias, dst=q_s, b_t=bq_s),
                )
                conv_proj(
                    wk_s, [s_s[:, k, :] for k in range(CT)], CT, CT,
                    partial(evac_bias, dst=k_s, b_t=bk_s),
                )

                # ---- v^T projection: vT[m, d] = s^T Wv^T, plus a ones column
                # so the attention matmul also yields the softmax denominator.
                vT = pairbuf.tile([P, MT, D + 1], bf16, tag="vT", bufs=2)
                for t in range(MT):
                    nc.vector.memset(vT[:, t, D : D + 1], 1.0)
                for t in range(MT):
                    ps = psum.tile([P, CHUNK], f32, tag="mm512", name="mmps")
                    for k in range(CT):
                        nc.tensor.matmul(
                            ps[:, 0:D],
                            s_s[:, k, t * P : (t + 1) * P],
                            wv_s[:, k, :],
                            start=(k == 0),
                            stop=(k == CT - 1),
                        )
                    nc.vector.tensor_copy(vT[:, t, 0:D], ps[:, 0:D])

                # ---- attention: S^T = k^T q (m on partitions), E = exp(S^T/16),
                # msg^T[n, 0:D] plus rowsum in col D via the ones column of vT.
                msgT = pairbuf.tile([P, MT, D], bf16, tag="msgT", bufs=2)
                msg_n = work.tile([P, CT, N], bf16, tag="msg_n", bufs=1)
                for j in range(NCH):
                    e_t = work.tile([P, MT, CHUNK], bf16, tag="e_t")
                    for t in range(MT):
                        ps = psum.tile([P, CHUNK], f32, tag="mm512", name="mmps")
                        for k in range(CT):
                            nc.tensor.matmul(
                                ps,
                                k_s[:, k, t * P : (t + 1) * P],
                                q_s[:, k, j * CHUNK : (j + 1) * CHUNK],
                                start=(k == 0),
                                stop=(k == CT - 1),
                            )
                        nc.scalar.activation(
                            e_t[:, t, :], ps, AF.Exp, scale=1.0 / 16.0
                        )
                    for u in range(NCH):
                        pm = psum_s.tile([P, D + 1], f32, tag="mm257")
                        for t in range(MT):
                            nc.tensor.matmul(
                                pm,
                                e_t[:, t, u * P : (u + 1) * P],
                                vT[:, t, :],
                                start=(t == 0),
                                stop=(t == MT - 1),
                            )
                        nsub = j * NCH + u
                        rec = work.tile([P, 1], f32, tag="rec")
                        nc.vector.reciprocal(rec, pm[:, D : D + 1])
                        nc.vector.tensor_scalar_mul(
                            msgT[:, nsub, :], pm[:, 0:D], rec
                        )
                        # transpose this n-subtile back to [D, n] on the
                        # DMA crossbar: no TensorE / PSUM / evacuation cost.
                        # (bv is folded into Wm's bias host-side.) The 3D out
                        # AP folds both channel tiles into one instruction.
                        nc.sync.dma_start_transpose(
                            out=msg_n[:, :, nsub * P : (nsub + 1) * P],
                            in_=msgT[:, nsub, :],
                        )

                # ---- Wm conv ----
                msg2 = work.tile([P, CT, N], bf16, tag="msg2")
                conv_proj(
                    wm_s, [msg_n[:, k, :] for k in range(CT)], CT, CT,
                    partial(evac_bias, dst=msg2, b_t=bm_s),
                )

                # ---- W1 over [x; msg2]. VectorE evacuates (bias b1);
                # ScalarE computes the BN sum-of-squares via Square+accum_out.
                # The plain BN sum is NOT accumulated per chunk: by linearity
                # sum_n h1 = W1 @ colsum([x; msg2]) + N*b1, computed below
                # from per-pair column sums (sigma) with tiny F=1 matmuls.
                sq_scr = work.tile([P, CHUNK], bf16, tag="sq_scr", bufs=1)
                w1_rhs = [x_s[:, 0, :], x_s[:, 1, :], msg2[:, 0, :], msg2[:, 1, :]]

                def evac_w1(m, j, ps):
                    slot = p * NCH + j
                    sl = slice(j * CHUNK, (j + 1) * CHUNK)
                    nc.vector.tensor_scalar_add(
                        h1[p][:, m, sl], ps, b1_s[:, m : m + 1]
                    )
                    nc.scalar.activation(
                        sq_scr,
                        ps,
                        AF.Square,
                        bias=b1_s[:, m : m + 1],
                        accum_out=ssq[:, m, slot : slot + 1],
                    )

                conv_proj(w1_s, w1_rhs, CT2, CT2, evac_w1)

                with nc.allow_low_precision(reason="bf16 colsums feed bf16 GEMM"):
                    for k in range(CT2):
                        nc.vector.reduce_sum(
                            sigma[:, p, k : k + 1],
                            w1_rhs[k],
                            axis=mybir.AxisListType.X,
                        )

            # ---- BN statistics: sum_n h1 = W1 @ sigma_total + N*b1 ----
            sig_t = persist.tile([P, CT2], bf16, tag="sig_t")
            nc.vector.tensor_add(sig_t, sigma[:, 0, :], sigma[:, 1, :])
            pstat = psum_s.tile([P, CT2], f32, tag="mm257", name="pstat")
            for m in range(CT2):
                for k in range(CT2):
                    nc.tensor.matmul(
                        pstat[:, m : m + 1],
                        w1_s[:, k, m * P : (m + 1) * P],
                        sig_t[:, k : k + 1],
                        start=(k == 0),
                        stop=(k == CT2 - 1),
                    )
            stats_l = persist.tile([P, 2 * CT2], f32, tag="stats_l")
            nb1 = persist.tile([P, CT2], f32, tag="nb1")
            nc.vector.tensor_scalar_mul(nb1, b1_s, float(2 * N))
            nc.vector.tensor_add(stats_l[:, 0:CT2], pstat, nb1)
            for m in range(CT2):
                nc.vector.reduce_sum(
                    stats_l[:, CT2 + m : CT2 + m + 1],
                    ssq[:, m, :],
                    axis=mybir.AxisListType.X,
                )
            # Cross-core reduction of the 4 KB BN stats via ncfw AllReduce.
            nc.sync.dma_start(out=cc_in[:], in_=stats_l[:])
            nc.gpsimd.collective_compute(
                "AllReduce",
                ALU.add,
                replica_groups=[list(range(NCORES))],
                ins=[cc_in[:].opt()],
                outs=[cc_out[:].opt()],
            )
            stats_g = persist.tile([P, 2 * CT2], f32, tag="stats_g")
            nc.sync.dma_start(out=stats_g[:], in_=cc_out[:])

            count = float(B * H * N)
            mom = persist.tile([P, 2 * CT2], f32, tag="mom")
            nc.vector.tensor_scalar_mul(mom, stats_g, 1.0 / count)
            var = persist.tile([P, CT2], f32, tag="var")
            nc.vector.tensor_mul(var, mom[:, 0:CT2], mom[:, 0:CT2])
            nc.vector.tensor_sub(var, mom[:, CT2 : 2 * CT2], var)
            # rsqrt(var + eps) = exp(-0.5 * ln(var + eps)) — same table set as
            # the attention Exp, so no mid-kernel ACT table switch.
            eps_t = persist.tile([P, 1], f32, tag="eps_t")
            nc.vector.memset(eps_t, EPS)
            lnv = persist.tile([P, CT2], f32, tag="lnv")
            nc.scalar.activation(lnv, var, AF.Ln, bias=eps_t)
            inv = persist.tile([P, CT2], f32, tag="inv")
            nc.scalar.activation(inv, lnv, AF.Exp, scale=-0.5)
            scl = persist.tile([P, CT2], f32, tag="scl")
            nc.vector.tensor_mul(scl, gm_s, inv)
            sft = persist.tile([P, CT2], f32, tag="sft")
            nc.vector.tensor_mul(sft, mom[:, 0:CT2], scl)
            nc.vector.tensor_sub(sft, bt_s, sft)

            # ---- pass 2: BN apply + ReLU (ScalarE), then W2 ----
            for p in range(PAIRS_PER_CORE):
                o_big = work.tile([P, CT, N], bf16, tag="o_big")
                for j in range(NCH):
                    h1n = work.tile([P, CT2, CHUNK], bf16, tag="h1n")
                    for m in range(CT2):
                        sl = slice(j * CHUNK, (j + 1) * CHUNK)
                        if m < 2:
                            nc.scalar.activation(
                                h1n[:, m, :],
                                h1[p][:, m, sl],
                                AF.Relu,
                                scale=scl[:, m : m + 1],
                                bias=sft[:, m : m + 1],
                            )
                        else:
                            nc.vector.tensor_scalar(
                                h1n[:, m, :],
                                h1[p][:, m, sl],
                                scl[:, m : m + 1],
                                sft[:, m : m + 1],
                                op0=ALU.mult,
                                op1=ALU.add,
                            )
                            nc.vector.tensor_scalar_max(
                                h1n[:, m, :], h1n[:, m, :], 0.0
                            )
                    for c in range(CT):
                        ps = psum.tile([P, CHUNK], f32, tag="mm512", name="mmps")
                        for k in range(CT2):
                            nc.tensor.matmul(
                                ps,
                                w2_s[:, k, c * P : (c + 1) * P],
                                h1n[:, k, :],
                                start=(k == 0),
                                stop=(k == CT2 - 1),
                            )
                        nc.vector.tensor_scalar_add(
                            o_big[:, c, j * CHUNK : (j + 1) * CHUNK],
                            ps,
                            b2_s[:, c : c + 1],
                        )
                for c in range(CT):
                    nc.sync.dma_start(out=out[p, c], in_=o_big[:, c, :])

    nc.finalize()
    return nc


def _get_nc():
    if "nc" not in _CACHE:
        _CACHE["nc"] = build_bass()
    return _CACHE["nc"]


def _prep_inputs(inputs):
    """Host-side shard/transpose/cast. Returns in_maps for the 8 cores."""
    x = np.asarray(inputs["x"], np.float32)
    source = np.asarray(inputs["source"], np.float32)

    # [B, D, H, N] -> [B*H pairs, P, CT, N] (partition-major for 1-shot DMA)
    def to_pairs(a):
        a = a.transpose(0, 2, 1, 3).reshape(B * H, CT, P, N)
        return np.ascontiguousarray(a.transpose(0, 2, 1, 3)).astype(BF16)

    xp = to_pairs(x)
    sp = to_pairs(source)

    def lhsT(w):
        # out = W @ r -> lhsT = W.T, laid out [P, CT_in, Cout] for 1-shot DMA
        wT = np.ascontiguousarray(np.asarray(w, np.float32).T)
        cin, cout = wT.shape
        a = wT.reshape(cin // P, P, cout).transpose(1, 0, 2)
        return np.ascontiguousarray(a).astype(BF16)

    def vcol(b):
        return np.asarray(b, np.float32).reshape(-1, P).T  # [P, kt]

    vecs = np.zeros((P, 24), np.float32)
    vecs[:, 0:2] = vcol(inputs["bq"])
    vecs[:, 2:4] = vcol(inputs["bk"])
    vecs[:, 4:6] = vcol(inputs["bv"])
    bm_eff = np.asarray(inputs["Wm"], np.float32) @ np.asarray(
        inputs["bv"], np.float32
    ) + np.asarray(inputs["bm"], np.float32)
    vecs[:, 6:8] = vcol(bm_eff)
    vecs[:, 8:12] = vcol(inputs["b1"])
    vecs[:, 12:14] = vcol(inputs["b2"])
    vecs[:, 14:18] = vcol(inputs["gamma"])
    vecs[:, 18:22] = vcol(inputs["beta"])

    common = {
        "wqT": lhsT(inputs["Wq"]),
        "wkT": lhsT(inputs["Wk"]),
        "wvT": lhsT(inputs["Wv"]),
        "wmT": lhsT(inputs["Wm"]),
        "w1T": lhsT(inputs["W1"]),
        "w2T": lhsT(inputs["W2"]),
        "vecs": vecs,
    }
    in_maps = []
    for i in range(NCORES):
        m = dict(common)
        m["xb"] = np.ascontiguousarray(xp[i * PAIRS_PER_CORE : (i + 1) * PAIRS_PER_CORE])
        m["sb"] = np.ascontiguousarray(sp[i * PAIRS_PER_CORE : (i + 1) * PAIRS_PER_CORE])
        in_maps.append(m)
    return in_maps


def run_on_hw(inputs, trace=False, **kw):
    nc = _get_nc()
    in_maps = _prep_inputs(inputs)
    res = run_bass_kernel_spmd(
        nc, in_maps, core_ids=list(range(NCORES)), trace=trace, **kw
    )
    outs = res.results
    full = np.empty((B, H, D, N), np.float32)
    for i in range(NCORES):
        o = np.asarray(outs[i]["out"]).astype(np.float32).reshape(PAIRS_PER_CORE, D, N)
        for jp in range(PAIRS_PER_CORE):
            gp = i * PAIRS_PER_CORE + jp
            full[gp // H, gp % H] = o[jp]
    return full.transpose(0, 2, 1, 3), res


def kernel(**inputs) -> np.ndarray:
    out, _ = run_on_hw(inputs, trace=False)
    return out

